# revision 1
# baseline (speedup 1.0000x reference)
"""BSRBF-KAN layer forward on 8 Trainium2 cores (Bass/Tile).

Math (per token t, output o):
    xn = LayerNorm(x) * g + b
    out[t,o] = sum_d relu(xn[t,d]) * Wb[o,d]
             + sum_{d,j} (B_j(xn[t,d]) + G_j(xn[t,d])) * Ws[o, d*8+j]

B_j: cardinal cubic B-spline on uniform knots (h=0.6, centers c_j=-2.1+0.6j):
    B_j(x) = [relu(2h-|x-c_j|)^3 - 4*relu(h-|x-c_j|)^3] / (6h^3)
computed by a fused custom DVE op (2 instructions per j):
    OP(x; s0,s1,imm2, src1) = max(min(s0-x, x-s1), 0)^3 * imm2 + src1
G_j: Gaussians exp(-((x-r_j)/D)^2), r_j uniform; anchors j in {0,4} via
ACT Square+Exp, the rest by the recurrence G_j = (G_{j-1}*c_j)*exp(d*x)
(one scalar_tensor_tensor each; algebraically exact).

The 9 feature channels (8 bsrbf + relu) feed a K=4608 fp32r matmul
(tokens as lhsT M-dim, 512 outputs as rhs N-dim), PSUM-accumulated.
Data-parallel: tokens sharded 8 ways, weights replicated.
"""

import numpy as np

# ---------------------------------------------------------------- constants
B, S, D, O = 4, 4096, 512, 512
TOKENS = B * S
CORES = 8
TPC = TOKENS // CORES          # tokens per core (2048)
NB = 8                         # basis funcs per input dim
H = 0.6                        # knot spacing
CJ = [-2.1 + 0.6 * j for j in range(NB)]   # spline centers
DELTA = 3.0 / 7.0              # rbf denom
RJ = [-1.5 + j * (3.0 / 7.0) for j in range(NB)]  # rbf centers
DLT = 2.0 * (3.0 / 7.0) / DELTA**2   # = 14/3, exponent scale of Q
LN_EPS = 1e-5
CUBE_SCALE = 1.0 / (6.0 * H**3)

# chain fold: channel_j = F_j + B_j/RHO[j], F_j = F_{j-1}*exp(DLT*x),
# weights scaled by RHO[j].  RHO = prod of per-step gaussian ratios.
_CC = {j: float(np.exp(-(3.0 / 7.0) * (RJ[j] + RJ[j - 1]) / DELTA**2))
       for j in (1, 2, 3, 5, 6, 7)}
DIRECT_J = (0, 1, 2, 4, 5, 6)   # rbf computed directly on ACT
RHO = [1.0] * NB
for _j in range(1, NB):
    RHO[_j] = 1.0 if _j in DIRECT_J else RHO[_j - 1] * _CC[_j]

BLK = 512                      # tokens per processing block
NBLK = TPC // BLK              # 4 blocks per core
QCH = D // 128                 # 4 d-chunks
NCH = NB + 1                   # 9 matmul channels per d-chunk
KT = QCH * NCH                 # 36 k-tiles

_BUILT = {}


# ------------------------------------------------------- custom DVE op
def _get_custom_op():
    """Register (idempotently) the fused spline-side op:
        out = max(min(s0 - in0, in0 - s1), 0)^3 * imm2 + in1
    """
    import concourse.dve_ops as dve_ops
    from concourse.dve_ops import DveOp
    from concourse.dve_spec import (
        Spec, Src0, Src1, C0, C1, C2, Zero, maxx, minn, sq, lower,
    )
    from concourse.dve_uop import DveOpSpec
    from concourse.dve_table_gen import dve_ver_for

    NAME = "BSPLINE_SIDE_ANT"
    have = {op.name: op for op in dve_ops.OPS}
    if NAME in have:
        return have[NAME], have["RSQRT_STEP_ANT"]

    hi = C0 - Src0
    lo = Src0 - C1
    m = maxx(minn(hi, lo), Zero)
    body = sq(m) * m * C2 + Src1

    def _ref(in0, in1, s0, s1, imm2):
        return (
            np.maximum(np.minimum(s0 - in0, in0 - s1), 0.0) ** 3 * imm2 + in1
        ).astype(np.float32)

    spec = Spec(body=body, reference=_ref)

    row = max(dve_ops._SUB_OPCODE_FOR_NAME.values()) + 1
    assert row < 0x20
    dve_ops._SUB_OPCODE_FOR_NAME[NAME] = row

    shas = {}
    for ver in ("v3", "v4"):
        try:
            uops = lower(spec, ver=ver)
            shas[ver] = DveOpSpec(name=NAME, opcode=row, uops=uops,
                                  rd1_en=True).sha(ver)
        except Exception:
            pass
    op = DveOp(NAME, spec, subdim=False, uops_sha=shas)
    dve_ops.OPS.append(op)
    dve_ops.CUSTOM_DVE_SPECS[NAME] = spec

    # rsqrt Newton step: out = y*(C0 - C1*v*y^2), y=Src0, v=Src1
    N2 = "RSQRT_STEP_ANT"
    body2 = Src0 * (C0 - C1 * Src1 * sq(Src0))

    def _ref2(in0, in1, s0, s1, imm2):
        return (in0 * (s0 - s1 * in1 * in0 * in0)).astype(np.float32)

    spec2 = Spec(body=body2, reference=_ref2)
    row2 = max(dve_ops._SUB_OPCODE_FOR_NAME.values()) + 1
    assert row2 < 0x20
    dve_ops._SUB_OPCODE_FOR_NAME[N2] = row2
    shas2 = {}
    for ver in ("v3", "v4"):
        try:
            uops2 = lower(spec2, ver=ver)
            shas2[ver] = DveOpSpec(name=N2, opcode=row2, uops=uops2,
                                   rd1_en=True).sha(ver)
        except Exception:
            pass
    op2 = DveOp(N2, spec2, subdim=False, uops_sha=shas2)
    dve_ops.OPS.append(op2)
    dve_ops.CUSTOM_DVE_SPECS[N2] = spec2
    return op, op2


# ------------------------------------------------------- bass program
def _build_program(matmul_dt_name="float32r", loop_n=None, ablate=None, nblk=None):
    import concourse.bass as bass
    import concourse.bacc as bacc
    import concourse.mybir as mybir
    import concourse.tile as tile
    from contextlib import ExitStack

    OPC, OPR = _get_custom_op()
    f32 = mybir.dt.float32
    mm_dt = getattr(mybir.dt, matmul_dt_name)
    AF = mybir.ActivationFunctionType
    ALU = mybir.AluOpType

    nc = bacc.Bacc("TRN2", target_bir_lowering=False, debug=False)
    xs = nc.declare_dram_parameter("xs", [TPC, D], f32, isOutput=False)
    wcat = nc.declare_dram_parameter("wcat", [KT * 128, O], mm_dt, isOutput=False)
    gmt = nc.declare_dram_parameter("gmt", [128, QCH], f32, isOutput=False)
    bet = nc.declare_dram_parameter("bet", [128, QCH], f32, isOutput=False)
    idn = nc.declare_dram_parameter("idn", [128, 128], f32, isOutput=False)
    out = nc.declare_dram_parameter("out", [TPC, O], f32, isOutput=True)

    def _register_const(val):
        key = (f32, float(val))
        if key not in nc.const_aps.aps:
            t = nc.alloc_sbuf_tensor(
                f"constf32_{len(nc.const_aps.aps)}", [128, 1], f32)
            nc.gpsimd.memset(t.ap(), float(val))
            nc.const_aps.aps[key] = t.ap()
    _register_const(LN_EPS)
    for j in DIRECT_J:
        _register_const(-RJ[j] / DELTA)
    nc.all_engine_barrier()

    with ExitStack() as ctx:
        tc = ctx.enter_context(tile.TileContext(nc))

        const_pool = ctx.enter_context(tc.tile_pool(name="const", bufs=1))
        w_pool = ctx.enter_context(tc.tile_pool(name="wts", bufs=1))
        x_pool = ctx.enter_context(tc.tile_pool(name="x", bufs=8))
        stat_pool = ctx.enter_context(tc.tile_pool(name="stat", bufs=10))
        xn_pool = ctx.enter_context(tc.tile_pool(name="xn", bufs=6))
        xnt_pool = ctx.enter_context(tc.tile_pool(name="xnt", bufs=6))
        rbf_pool = ctx.enter_context(tc.tile_pool(name="rbf", bufs=6))
        q_pool = ctx.enter_context(tc.tile_pool(name="q", bufs=3))
        t1_pool = ctx.enter_context(tc.tile_pool(name="t1", bufs=4))
        feat_pool = ctx.enter_context(tc.tile_pool(name="feat", bufs=6))
        relu_pool = ctx.enter_context(tc.tile_pool(name="relu", bufs=2))
        osb_pool = ctx.enter_context(tc.tile_pool(name="osb", bufs=6))
        tp_psum = ctx.enter_context(tc.tile_pool(name="tpp", bufs=2, space="PSUM"))
        out_psum = ctx.enter_context(tc.tile_pool(name="opp", bufs=4, space="PSUM"))

        # --- constants / weights to SBUF
        ident = const_pool.tile([128, 128], f32, tag="ident")
        nc.sync.dma_start(ident[:], idn[:, :])
        gam = const_pool.tile([128, QCH], f32, tag="gam")
        nc.sync.dma_start(gam[:], gmt[:, :])
        bta = const_pool.tile([128, QCH], f32, tag="bta")
        nc.sync.dma_start(bta[:], bet[:, :])

        wt = []
        for kt in range(KT):
            w = w_pool.tile([128, O], mm_dt, tag=f"w{kt}")
            nc.sync.dma_start(w[:], wcat[kt * 128:(kt + 1) * 128, :])
            wt.append(w)

        def _emit_blocks():
            for blk in range(nblk or NBLK):
                # ---- load + layernorm, 4 token-tiles of [128, D]
                xts, mvs = [], []
                vb = stat_pool.tile([128, 4], f32, tag="vb",
                                    name=f"vb{blk}")
                for i in range(4):
                    t0 = blk * BLK + i * 128
                    xt = x_pool.tile([128, D], f32)
                    nc.sync.dma_start(xt[:], xs[t0:t0 + 128, :])
                    st6 = stat_pool.tile([128, 6], f32, tag="st6")
                    nc.vector.bn_stats(st6[:], xt[:])
                    mv = stat_pool.tile([128, 2], f32, tag="mv",
                                        name=f"mv{blk}_{i}")
                    nc.vector.bn_aggr(mv[:], st6[:])
                    nc.vector.tensor_scalar(
                        vb[:, i:i + 1], mv[:, 1:2], LN_EPS, None, op0=ALU.add)
                    xts.append(xt)
                    mvs.append(mv)
                # rstd = rsqrt(vb) via linear seed + 4 Newton steps (batched)
                yb = stat_pool.tile([128, 4], f32, tag="yb", name=f"yb{blk}")
                nc.vector.tensor_scalar(yb[:], vb[:], -0.5, 1.5,
                                        op0=ALU.mult, op1=ALU.add)
                for _ in range(4):
                    yn = stat_pool.tile([128, 4], f32, tag="yb",
                                        name=f"yn{blk}_{_}")
                    nc.vector._custom_dve(OPR, out=yn[:], in0=yb[:],
                                          in1=vb[:], s0=1.5, s1=0.5)
                    yb = yn
                xn_tiles = []
                for i in range(4):
                    xnt_ = xn_pool.tile([128, D], f32)
                    nc.vector.tensor_scalar(
                        xnt_[:], xts[i][:], mvs[i][:, 0:1], yb[:, i:i + 1],
                        op0=ALU.subtract, op1=ALU.mult)
                    xn_tiles.append(xnt_)

                # ---- transpose to [128d, BLK t] per d-chunk, apply gamma/beta
                xnT = []
                for q in range(QCH):
                    pt = tp_psum.tile([128, BLK], f32, tag="pt",
                                      name=f"pt{blk}_{q}")
                    for i in range(4):
                        nc.tensor.transpose(
                            pt[:, i * 128:(i + 1) * 128],
                            xn_tiles[i][:, q * 128:(q + 1) * 128],
                            ident[:])
                    xq = xnt_pool.tile([128, BLK], f32, tag="xq",
                                       name=f"xq{blk}_{q}")
                    nc.scalar.activation(
                        xq[:], pt[:], AF.Identity,
                        bias=bta[:, q:q + 1], scale=gam[:, q:q + 1])
                    xnT.append(xq)

                # ---- features + matmuls per d-chunk
                po = [out_psum.tile([128, O], f32, tag="po", name=f"po{blk}_{m}")
                      for m in range(4)]
                for q in range(QCH):
                    xq = xnT[q]
                    qt = q_pool.tile([128, BLK], f32, tag="qt", name=f"qt{blk}_{q}")
                    nc.scalar.activation(qt[:], xq[:], AF.Exp, scale=DLT)
                    rl = relu_pool.tile([128, BLK], mm_dt, tag="rl",
                                        name=f"rl{blk}_{q}")
                    nc.scalar.activation(rl[:], xq[:], AF.Relu)

                    feats = []
                    rbf = {}
                    for j in range(NB):
                        r = rbf_pool.tile([128, BLK], f32, tag="rbf",
                                          name=f"rbf{blk}_{q}_{j}")
                        if j in DIRECT_J:
                            z2 = rbf_pool.tile([128, BLK], f32, tag="z2",
                                               name=f"z2{blk}_{q}_{j}",
                                               bufs=3)
                            nc.scalar.activation(
                                z2[:], xq[:], AF.Square,
                                bias=-RJ[j] / DELTA, scale=1.0 / DELTA)
                            nc.scalar.activation(r[:], z2[:], AF.Exp, scale=-1.0)
                        else:
                            nc.vector.tensor_tensor(
                                r[:], rbf[j - 1][:], qt[:], op=ALU.mult)
                        rbf[j] = r
                        t1 = t1_pool.tile([128, BLK], f32, tag="t1",
                                          name=f"t1{blk}_{q}_{j}")
                        nc.vector._custom_dve(
                            OPC, out=t1[:], in0=xq[:], in1=rbf[j][:],
                            s0=CJ[j] + 2 * H, s1=CJ[j] - 2 * H,
                            imm2=CUBE_SCALE / RHO[j])
                        bs = feat_pool.tile([128, BLK], mm_dt, tag="bsrbf",
                                            name=f"bs{blk}_{q}_{j}")
                        nc.vector._custom_dve(
                            OPC, out=bs[:], in0=xq[:], in1=t1[:],
                            s0=CJ[j] + H, s1=CJ[j] - H,
                            imm2=-4.0 * CUBE_SCALE / RHO[j])
                        feats.append(bs)
                    feats.append(rl)

                    for ch in range(NCH):
                        f = feats[ch]
                        w = wt[q * NCH + ch]
                        if ablate == "nomm":
                            if q == 0 and ch == 0:
                                for m in range(4):
                                    nc.tensor.matmul(
                                        po[m][:], f[:, m * 128:(m + 1) * 128],
                                        w[:], start=True, stop=True)
                            continue
                        for m in range(4):
                            nc.tensor.matmul(
                                po[m][:],
                                f[:, m * 128:(m + 1) * 128],
                                w[:],
                                start=(q == 0 and ch == 0),
                                stop=(q == QCH - 1 and ch == NCH - 1))

                # ---- evacuate + store
                for m in range(4):
                    ot = osb_pool.tile([128, O], f32, tag="ot",
                                       name=f"ot{blk}_{m}")
                    nc.scalar.copy(ot[:], po[m][:])
                    t0 = blk * BLK + m * 128
                    nc.sync.dma_start(out[t0:t0 + 128, :], ot[:])

        from contextlib import nullcontext
        loop_cm = tc.For_i(0, loop_n, 1) if loop_n else nullcontext()
        with loop_cm:
            _emit_blocks()

    nc.compile()
    return nc
def _host_prep(x, ln_weight, ln_bias, base_weight, spline_weight):
    x = np.ascontiguousarray(np.asarray(x, dtype=np.float32)).reshape(TOKENS, D)
    ln_weight = np.asarray(ln_weight, dtype=np.float32)
    ln_bias = np.asarray(ln_bias, dtype=np.float32)
    base_weight = np.asarray(base_weight, dtype=np.float32)
    spline_weight = np.asarray(spline_weight, dtype=np.float32)

    # wcat[(q*9+ch)*128 + dl, o]
    wsp = spline_weight.reshape(O, D, NB)          # [o, d, j]
    blocks = np.empty((QCH, NCH, 128, O), dtype=np.float32)
    wsp_t = np.transpose(wsp, (1, 2, 0))            # [d, j, o]
    rho = np.asarray(RHO, dtype=np.float64)[:, None, None]
    for q in range(QCH):
        blocks[q, :NB] = (np.transpose(
            wsp_t[q * 128:(q + 1) * 128], (1, 0, 2)).astype(np.float64)
            * rho).astype(np.float32)  # [j, dl, o]
        blocks[q, NB] = base_weight.T[q * 128:(q + 1) * 128]
    wcat = np.ascontiguousarray(blocks.reshape(KT * 128, O))

    gmt = np.ascontiguousarray(ln_weight.reshape(QCH, 128).T)
    bet = np.ascontiguousarray(ln_bias.reshape(QCH, 128).T)
    idn = np.eye(128, dtype=np.float32)
    return x, wcat, gmt, bet, idn


def kernel(x, ln_weight, ln_bias, base_weight, spline_weight):
    from concourse.bass_utils import run_bass_kernel_spmd

    if "nc" not in _BUILT:
        _BUILT["nc"] = _build_program()
    nc = _BUILT["nc"]

    xf, wcat, gmt, bet, idn = _host_prep(
        x, ln_weight, ln_bias, base_weight, spline_weight)

    in_maps = []
    for c in range(CORES):
        in_maps.append({
            "xs": np.ascontiguousarray(xf[c * TPC:(c + 1) * TPC]),
            "wcat": wcat, "gmt": gmt, "bet": bet, "idn": idn,
        })
    res = run_bass_kernel_spmd(nc, in_maps, core_ids=list(range(CORES)))
    outs = [res.results[c]["out"] for c in range(CORES)]
    full = np.concatenate(outs, axis=0).reshape(B, S, O)
    return full.astype(np.float32)

